# revision 10
# baseline (speedup 1.0000x reference)
"""Multi-headed attention (B=4, S=2048, D=1024, H=16) on 8 trn2 NeuronCores.

Sharding: core c handles batch b=c//2, head-half hh=c%2 (heads hh*8..hh*8+7).
Host casts x/weights to bf16 (halves DMA, removes on-chip staging casts).

Per core:
  phase 1: K projection with N=2048 streams (ft-major, ft0/ft1 first so
           attention can start after 2 of 4 feature tiles), Qt for the
           first pair, V chunks produced just-in-time as PE filler.
  phase 2: per (t = 512 queries, jg = 2 head-pairs = 4 heads):
           per key-chunk k: scores for pair A (row-split concurrent pair)
           -> exp on ACT ([128, 2, 512] psum ping-pong so ACT never
           starves), same for pair B, AV as column-tiled pairs (two M=64
           matmuls at col offsets 0/64 share one 512-cycle slot), row sums
           as 4-way column-tiled M=1 matmuls (strips 0/32/64/96).
           Normalization: reciprocal of sums, K=1 broadcast matmuls
           (col-tiled), fused into the X drain. Q/V/O projections emitted
           as fillers inside the ACT-bound window.
Host: out[b] = core(2b) + core(2b+1) + bo.
"""

import numpy as np
import ml_dtypes

import concourse.tile as tile
from concourse import bacc, mybir
from concourse.bass_utils import run_bass_kernel_spmd

B, S, D, H = 4, 2048, 1024, 16
HD = D // 2          # feature columns per core (8 heads * 64)
KC = D // 128        # 8 contraction chunks over model dim
FT = HD // 128       # 4 feature tiles (head pairs)
ST = S // 512        # 4 query tiles
RT = S // 128        # 16 row tiles / S_k chunks

f32 = mybir.dt.float32
bf16 = mybir.dt.bfloat16
EXP = mybir.ActivationFunctionType.Exp

_CACHED_NC = None
_LAST_IN_MAPS = None


def build_nc():
    nc = bacc.Bacc("TRN2", target_bir_lowering=False, debug=False)

    xq_d = nc.dram_tensor("xq", (D, S), bf16, kind="ExternalInput")
    xk_d = nc.dram_tensor("xk", (D, S), bf16, kind="ExternalInput")
    xv_d = nc.dram_tensor("xv", (D, S), bf16, kind="ExternalInput")
    wq_d = nc.dram_tensor("wq", (D, HD), bf16, kind="ExternalInput")
    wk_d = nc.dram_tensor("wk", (D, HD), bf16, kind="ExternalInput")
    wv_d = nc.dram_tensor("wv", (D, HD), bf16, kind="ExternalInput")
    wo_d = nc.dram_tensor("wo", (HD, D), bf16, kind="ExternalInput")
    bqr_d = nc.dram_tensor("bqr", (128, FT), f32, kind="ExternalInput")
    bkr_d = nc.dram_tensor("bkr", (128, FT), f32, kind="ExternalInput")
    bv_d = nc.dram_tensor("bv", (1, HD), bf16, kind="ExternalInput")
    o_d = nc.dram_tensor("o", (S, D), f32, kind="ExternalOutput")

    with tile.TileContext(nc) as tc:
        with (
            tc.tile_pool(name="cpool", bufs=1) as cpool,
            tc.tile_pool(name="big", bufs=1) as big,
        ):
            # constants
            ones_f = cpool.tile([128, 128], f32, name="ones_f")
            nc.gpsimd.memset(ones_f[:], 1.0)
            ones_b = cpool.tile([128, 128], bf16, name="ones_b")
            nc.vector.tensor_copy(ones_b[:], ones_f[:])

            bqr_s = cpool.tile([128, FT], f32, name="bqr_s")
            nc.sync.dma_start(bqr_s[:], bqr_d[:])
            bkr_s = cpool.tile([128, FT], f32, name="bkr_s")
            nc.sync.dma_start(bkr_s[:], bkr_d[:])
            bv_r = cpool.tile([1, HD], bf16, name="bv_r")
            nc.sync.dma_start(bv_r[:], bv_d[:])

            # persistent SBUF arrays
            K = big.tile([128, FT, S], bf16, name="Kfm")
            Vs = big.tile([128, RT, 8, 64], bf16, name="Vs")
            X = big.tile([128, FT, S], bf16, name="Xfm")
            wk_s = big.tile([128, KC, HD], bf16, name="wk_s")
            wv_s = big.tile([128, KC, HD], bf16, name="wv_s")
            wq_s = big.tile([128, KC, HD], bf16, name="wq_s")
            wo_s = big.tile([128, FT, D], bf16, name="wo_s")
            xk_sb = big.tile([128, KC, S], bf16, name="xk_sb")
            xv_sb = big.tile([128, KC, S], bf16, name="xv_sb")

            # weight/x DMAs. sync queue: K-projection critical path.
            wk_src = wk_d[:].rearrange("(k p) n -> p k n", p=128)
            for kc in range(KC):
                nc.sync.dma_start(wk_s[:, kc, :], wk_src[:, kc, :])
            xk_src = xk_d[:].rearrange("(k p) s -> p k s", p=128)
            xv_src = xv_d[:].rearrange("(k p) s -> p k s", p=128)
            for kc in range(KC):
                eng = nc.sync if kc % 2 == 0 else nc.gpsimd
                eng.dma_start(xk_sb[:, kc, :], xk_src[:, kc, :])
            # gpsimd queue: wq + xq(t0) early (Qt0 needed at attention start)
            wq_src = wq_d[:].rearrange("(k p) n -> p k n", p=128)
            for kc in range(KC):
                nc.gpsimd.dma_start(wq_s[:, kc, :], wq_src[:, kc, :])
            wv_src = wv_d[:].rearrange("(k p) n -> p k n", p=128)
            for kc in range(KC):
                nc.gpsimd.dma_start(wv_s[:, kc, :], wv_src[:, kc, :])
            for kc in range(KC):
                eng = nc.sync if kc % 2 == 0 else nc.gpsimd
                eng.dma_start(xv_sb[:, kc, :], xv_src[:, kc, :])
            wo_src = wo_d[:].rearrange("(f p) n -> p f n", p=128)
            for fc in range(FT):
                nc.gpsimd.dma_start(wo_s[:, fc, :], wo_src[:, fc, :])

            with (
                tc.tile_pool(name="xqp", bufs=2) as xqp,
                tc.tile_pool(name="qtp", bufs=4) as qtp,
                tc.tile_pool(name="ptp", bufs=6) as ptp,
                tc.tile_pool(name="rivp", bufs=2) as rivp,
                tc.tile_pool(name="ostage", bufs=4) as ostage,
            ):
                xq_src = xq_d[:].rearrange("(k p) s -> p k s", p=128)

                def stage_xq(t):
                    xt = xqp.tile([128, KC, 512], bf16, tag="xq", name="xqt")
                    tsl = slice(t * 512, (t + 1) * 512)
                    for kc in range(KC):
                        nc.sync.dma_start(xt[:, kc, :], xq_src[:, kc, tsl])
                    return xt

                xq_tiles = {0: stage_xq(0)}

                # ---------------- phase 1: K projection ----------------
                # (matmul free dim caps at 512: one f32 PSUM bank)
                with tc.tile_pool(name="pk1", bufs=4, space="PSUM") as pk1:
                    for ft in range(FT):
                        for h in range(4):
                            hsl = slice(h * 512, (h + 1) * 512)
                            ps = pk1.tile([128, 512], f32, tag="pk",
                                          name="pk")
                            for kc in range(KC):
                                nc.tensor.matmul(
                                    ps[:],
                                    wk_s[:, kc, ft * 128 : (ft + 1) * 128],
                                    xk_sb[:, kc, hsl],
                                    start=(kc == 0),
                                    stop=(kc == KC - 1),
                                )
                            nc.vector.tensor_scalar_add(
                                K[:, ft, hsl], ps[:], bkr_s[:, ft : ft + 1]
                            )

                # ---------------- phase 2 pools ----------------
                with (
                    tc.tile_pool(name="psc", bufs=2, space="PSUM") as psc,
                    tc.tile_pool(name="pav", bufs=2, space="PSUM") as pav,
                    tc.tile_pool(name="prs", bufs=1, space="PSUM") as prs,
                    tc.tile_pool(name="pqo", bufs=1, space="PSUM") as pqo,
                ):
                    def emit_qt(t, j):
                        qp = pqo.tile([128, 512], f32, tag="qo", name="qp")
                        xt = xq_tiles[t]
                        for kc in range(KC):
                            nc.tensor.matmul(
                                qp[:],
                                wq_s[:, kc, j * 128 : (j + 1) * 128],
                                xt[:, kc, :],
                                start=(kc == 0),
                                stop=(kc == KC - 1),
                            )
                        qt = qtp.tile([128, 512], bf16, tag="qt", name="qt")
                        nc.vector.tensor_scalar_add(
                            qt[:], qp[:], bqr_s[:, j : j + 1]
                        )
                        return qt

                    def emit_vproj(rt):
                        ps = pqo.tile([128, 512], f32, tag="qo", name="vp")
                        for kc in range(KC):
                            nc.tensor.matmul(
                                ps[:],
                                xv_sb[:, kc, rt * 128 : (rt + 1) * 128],
                                wv_s[:, kc, :],
                                start=(kc == 0),
                                stop=False,
                            )
                        nc.tensor.matmul(
                            ps[:],
                            ones_b[0:1, :],
                            bv_r[0:1, :],
                            start=False,
                            stop=True,
                        )
                        nc.vector.tensor_copy(
                            Vs[:, rt, :, :],
                            ps[:].rearrange("p (h e) -> p h e", h=8),
                        )

                    def emit_outproj_piece(t, r2, n):
                        rt = t * 4 + r2
                        rsl = slice(rt * 128, (rt + 1) * 128)
                        ps = pqo.tile([128, 512], f32, tag="qo", name="po")
                        for fc in range(FT):
                            nc.tensor.matmul(
                                ps[:],
                                X[:, fc, rsl],
                                wo_s[:, fc, n * 512 : (n + 1) * 512],
                                start=(fc == 0),
                                stop=(fc == FT - 1),
                            )
                        ot = ostage.tile([128, 512], f32, tag="os", name="os")
                        nc.vector.tensor_copy(ot[:], ps[:])
                        nc.sync.dma_start(
                            o_d[rsl, n * 512 : (n + 1) * 512], ot[:]
                        )

                    # Qt for (t0, pairs 0/1); V chunks via fillers below
                    qts = {(0, 0): emit_qt(0, 0), (0, 1): emit_qt(0, 1)}

                    for t in range(ST):
                        tsl = slice(t * 512, (t + 1) * 512)
                        for jg in range(2):
                            jA, jB = 2 * jg, 2 * jg + 1
                            qtA = qts.pop((t, jA))
                            qtB = qts.pop((t, jB))
                            xab = pav.tile([128, 512], f32, tag="av", name="xab")
                            xcd = pav.tile([128, 512], f32, tag="av", name="xcd")
                            rs = prs.tile([128, 512], f32, tag="rs", name="rs")

                            # filler work queue for this jg.  V-proj for
                            # t0/jg0 is NOT a filler: each chunk must be
                            # emitted before the AV that consumes it (the
                            # tile framework orders deps by program order).
                            fillers = []
                            if jg == 0 and t > 0:
                                for r2 in range(4):
                                    for n in range(2):
                                        fillers.append(
                                            ("op", (t - 1, r2, n)))
                            if jg == 0:
                                fillers.append(("qt", (t, 2)))
                                fillers.append(("qt", (t, 3)))
                                if t < ST - 1:
                                    fillers.append(("xq", t + 1))
                            elif t < ST - 1:
                                fillers.append(("qt", (t + 1, 0)))
                                fillers.append(("qt", (t + 1, 1)))

                            def pop_filler():
                                if not fillers:
                                    return
                                kind, arg = fillers.pop(0)
                                if kind == "qt":
                                    qts[arg] = emit_qt(*arg)
                                elif kind == "xq":
                                    xq_tiles[arg] = stage_xq(arg)
                                elif kind == "op":
                                    emit_outproj_piece(*arg)

                            if t == 0 and jg == 0:
                                for rt in range(3):
                                    emit_vproj(rt)

                            for k in range(RT):
                                ksl = slice(k * 128, (k + 1) * 128)
                                # pair A scores + exp
                                sAB = psc.tile([128, 2, 512], f32, tag="sc",
                                               name="sAB")
                                nc.tensor.matmul(
                                    sAB[:, 0, :], K[0:64, jA, ksl],
                                    qtA[0:64, :],
                                    start=True, stop=True,
                                    tile_position=(0, 0),
                                )
                                nc.tensor.matmul(
                                    sAB[:, 1, :], K[64:128, jA, ksl],
                                    qtA[64:128, :],
                                    start=True, stop=True,
                                    tile_position=(64, 0),
                                )
                                pAB = ptp.tile([128, 2, 512], bf16, tag="pt",
                                               name="pAB")
                                nc.scalar.activation(
                                    pAB[:], sAB[:], EXP, scale=0.125)
                                # pair B scores + exp
                                sCD = psc.tile([128, 2, 512], f32, tag="sc",
                                               name="sCD")
                                nc.tensor.matmul(
                                    sCD[:, 0, :], K[0:64, jB, ksl],
                                    qtB[0:64, :],
                                    start=True, stop=True,
                                    tile_position=(0, 0),
                                )
                                nc.tensor.matmul(
                                    sCD[:, 1, :], K[64:128, jB, ksl],
                                    qtB[64:128, :],
                                    start=True, stop=True,
                                    tile_position=(64, 0),
                                )
                                pCD = ptp.tile([128, 2, 512], bf16, tag="pt",
                                               name="pCD")
                                nc.scalar.activation(
                                    pCD[:], sCD[:], EXP, scale=0.125)

                                # V chunks just-in-time, 3 ahead of the AV
                                if t == 0 and jg == 0 and k + 3 < RT:
                                    emit_vproj(k + 3)

                                # AV: column-tiled pairs (M=64 at cols 0/64)
                                st = (k == 0)
                                sp = (k == RT - 1)
                                nc.tensor.matmul(
                                    xab[0:64, :], Vs[:, k, 2 * jA, :],
                                    pAB[:, 0, :], start=st, stop=sp,
                                    tile_position=(0, 0),
                                    skip_group_check=True,
                                )
                                nc.tensor.matmul(
                                    xab[64:128, :], Vs[:, k, 2 * jA + 1, :],
                                    pAB[:, 1, :], start=st, stop=sp,
                                    tile_position=(0, 64),
                                    skip_group_check=True,
                                )
                                nc.tensor.matmul(
                                    xcd[0:64, :], Vs[:, k, 2 * jB, :],
                                    pCD[:, 0, :], start=st, stop=sp,
                                    tile_position=(0, 0),
                                    skip_group_check=True,
                                )
                                nc.tensor.matmul(
                                    xcd[64:128, :], Vs[:, k, 2 * jB + 1, :],
                                    pCD[:, 1, :], start=st, stop=sp,
                                    tile_position=(0, 64),
                                    skip_group_check=True,
                                )
                                # row sums: 4-way column-tiled M=1
                                for idx, pr in enumerate(
                                    (pAB[:, 0, :], pAB[:, 1, :],
                                     pCD[:, 0, :], pCD[:, 1, :])
                                ):
                                    nc.tensor.matmul(
                                        rs[32 * idx : 32 * idx + 1, :],
                                        ones_b[:, 0:1], pr,
                                        start=st, stop=sp,
                                        tile_position=(0, 32 * idx),
                                        skip_group_check=True,
                                    )

                                pop_filler()

                            while fillers:
                                pop_filler()

                            # normalize: reciprocal + K=1 broadcast matmuls
                            rinv = rivp.tile([128, 512], f32, tag="ri",
                                             name="rinv")
                            nc.vector.reciprocal_approx_fast(rinv[:], rs[:])
                            bcAB = pqo.tile([128, 512], f32, tag="qo",
                                            name="bcAB")
                            nc.tensor.matmul(
                                bcAB[0:64, :], ones_f[0:1, 0:64],
                                rinv[0:1, :], start=True, stop=True,
                                tile_position=(0, 0),
                                skip_group_check=True,
                            )
                            nc.tensor.matmul(
                                bcAB[64:128, :], ones_f[32:33, 0:64],
                                rinv[32:33, :], start=True, stop=True,
                                tile_position=(32, 64),
                                skip_group_check=True,
                            )
                            nc.vector.tensor_copy(X[:, jA, tsl], xab[:])
                            nc.vector.tensor_mul(
                                X[:, jA, tsl], X[:, jA, tsl], bcAB[:])
                            bcCD = pqo.tile([128, 512], f32, tag="qo",
                                            name="bcCD")
                            nc.tensor.matmul(
                                bcCD[0:64, :], ones_f[64:65, 0:64],
                                rinv[64:65, :], start=True, stop=True,
                                tile_position=(64, 0),
                                skip_group_check=True,
                            )
                            nc.tensor.matmul(
                                bcCD[64:128, :], ones_f[96:97, 0:64],
                                rinv[96:97, :], start=True, stop=True,
                                tile_position=(96, 64),
                                skip_group_check=True,
                            )
                            nc.vector.tensor_copy(X[:, jB, tsl], xcd[:])
                            nc.vector.tensor_mul(
                                X[:, jB, tsl], X[:, jB, tsl], bcCD[:])

                    # final out projection for t=3
                    for r2 in range(4):
                        for n in range(2):
                            emit_outproj_piece(ST - 1, r2, n)

    nc.compile()
    return nc


def kernel(**inputs):
    global _CACHED_NC, _LAST_IN_MAPS
    if _CACHED_NC is None:
        _CACHED_NC = build_nc()
    nc = _CACHED_NC

    bf = ml_dtypes.bfloat16
    query = np.asarray(inputs["query"], dtype=np.float32)
    key = np.asarray(inputs["key"], dtype=np.float32)
    value = np.asarray(inputs["value"], dtype=np.float32)
    fc_w = np.asarray(inputs["fc_w"], dtype=np.float32)
    Wq = np.asarray(inputs["Wq"], dtype=np.float32)
    Wk = np.asarray(inputs["Wk"], dtype=np.float32)
    Wv = np.asarray(inputs["Wv"], dtype=np.float32)
    Wo = np.asarray(inputs["Wo"], dtype=np.float32)
    bq = np.asarray(inputs["bq"], dtype=np.float32)
    bk = np.asarray(inputs["bk"], dtype=np.float32)
    bv = np.asarray(inputs["bv"], dtype=np.float32)
    bo = np.asarray(inputs["bo"], dtype=np.float32)

    wq_eff = (fc_w * Wq).astype(bf)
    wk_b = Wk.astype(bf)
    wv_b = Wv.astype(bf)
    wo_b = Wo.astype(bf)
    xs = {}
    for b in range(B):
        xs[b] = (
            np.ascontiguousarray(query[b].T.astype(bf)),
            np.ascontiguousarray(key[b].T.astype(bf)),
            np.ascontiguousarray(value[b].T.astype(bf)),
        )

    in_maps = []
    for c in range(8):
        b, hh = c // 2, c % 2
        hs = slice(hh * HD, (hh + 1) * HD)
        xq, xk, xv = xs[b]
        in_maps.append({
            "xq": xq,
            "xk": xk,
            "xv": xv,
            "wq": np.ascontiguousarray(wq_eff[:, hs]),
            "wk": np.ascontiguousarray(wk_b[:, hs]),
            "wv": np.ascontiguousarray(wv_b[:, hs]),
            "wo": np.ascontiguousarray(wo_b[hs, :]),
            "bqr": np.ascontiguousarray(bq[hs].reshape(FT, 128).T),
            "bkr": np.ascontiguousarray(bk[hs].reshape(FT, 128).T),
            "bv": bv[None, hs].astype(bf),
        })

    _LAST_IN_MAPS = in_maps
    res = run_bass_kernel_spmd(nc, in_maps, core_ids=list(range(8)))

    out = np.empty((B, S, D), dtype=np.float32)
    for b in range(B):
        out[b] = res.results[2 * b]["o"] + res.results[2 * b + 1]["o"] + bo
    return out


# revision 27
# speedup vs baseline: 1.3349x; 1.3349x over previous
"""Multi-headed attention (B=4, S=2048, D=1024, H=16) on 8 trn2 NeuronCores.

Sharding: core c handles batch b=c//2, head-half hh=c%2 (heads hh*8..hh*8+7).
Host casts x/weights to bf16 (halves DMA, removes on-chip staging casts).

Per core:
  phase 1: xk DMA + K projection ft0 only, V chunks 0-2, Qt(0,0) -- then
           attention starts; K ft1-3, V 3-15, later Qt / out-projections
           are emitted as PE filler inside the ACT-bound attention window.
  phase 2: per (t = 512 queries, j = head pair): 16 chunk-steps.  Step k:
           scores for both heads of the pair as a row-split concurrent
           matmul pair into one [128, 2(head), 512] psum tile (pool
           bufs=2 ping-pong so ACT never starves), one exp ACTIVATE
           (1024 free elems), two M=65 AV matmuls (V augmented with a
           ones column -> unnormalized X and row sums in one pass).
           Normalization: reciprocal of the row-sum rows, K=1 broadcast
           matmuls (f32, col-paired), fused into the X drain.
Host: out[b] = core(2b) + core(2b+1) + bo.
"""

import numpy as np
import ml_dtypes

import concourse.tile as tile
from concourse import bacc, mybir
from concourse.bass_utils import run_bass_kernel_spmd

B, S, D, H = 4, 2048, 1024, 16
HD = D // 2          # feature columns per core (8 heads * 64)
KC = D // 128        # 8 contraction chunks over model dim
FT = HD // 128       # 4 feature tiles (head pairs)
ST = S // 512        # 4 query tiles
RT = S // 128        # 16 row tiles / S_k chunks

f32 = mybir.dt.float32
bf16 = mybir.dt.bfloat16
EXP = mybir.ActivationFunctionType.Exp

_CACHED_NC = None
_LAST_IN_MAPS = None
DEBUG_DUMPS = False


def build_nc():
    nc = bacc.Bacc("TRN2", target_bir_lowering=False, debug=False)

    xq_d = nc.dram_tensor("xq", (D, S), bf16, kind="ExternalInput")
    xk_d = nc.dram_tensor("xk", (D, S), bf16, kind="ExternalInput")
    xv_d = nc.dram_tensor("xv", (D, S), bf16, kind="ExternalInput")
    wq_d = nc.dram_tensor("wq", (D, HD), bf16, kind="ExternalInput")
    wk_d = nc.dram_tensor("wk", (D, HD), bf16, kind="ExternalInput")
    wv_d = nc.dram_tensor("wv", (D, HD), bf16, kind="ExternalInput")
    wo_d = nc.dram_tensor("wo", (HD, D), bf16, kind="ExternalInput")
    bqr_d = nc.dram_tensor("bqr", (128, FT), f32, kind="ExternalInput")
    bkr_d = nc.dram_tensor("bkr", (128, FT), f32, kind="ExternalInput")
    bv_d = nc.dram_tensor("bv", (1, HD), bf16, kind="ExternalInput")
    o_d = nc.dram_tensor("o", (S, D), f32, kind="ExternalOutput")
    if DEBUG_DUMPS:
        dbg_k = nc.dram_tensor("dbg_k", (128, FT * S), bf16,
                               kind="ExternalOutput")
        dbg_v = nc.dram_tensor("dbg_v", (128, RT * 8 * 65), bf16,
                               kind="ExternalOutput")
        dbg_x = nc.dram_tensor("dbg_x", (128, FT * S), bf16,
                               kind="ExternalOutput")
        dbg_qt = nc.dram_tensor("dbg_qt", (128, 512), bf16,
                                kind="ExternalOutput")
        dbg_ri = nc.dram_tensor("dbg_ri", (2, 512), f32,
                                kind="ExternalOutput")
        dbg_rs = nc.dram_tensor("dbg_rs", (2, 512), f32,
                                kind="ExternalOutput")
        dbg_p = nc.dram_tensor("dbg_p", (128, 2, 512), bf16,
                               kind="ExternalOutput")
        dbg_xk = nc.dram_tensor("dbg_xk", (128, KC * S), bf16,
                                kind="ExternalOutput")
        dbg_wk = nc.dram_tensor("dbg_wk", (128, KC * HD), bf16,
                                kind="ExternalOutput")

    with tile.TileContext(nc) as tc:
        with (
            tc.tile_pool(name="cpool", bufs=1) as cpool,
            tc.tile_pool(name="big", bufs=1) as big,
        ):
            # constants
            ones_f = cpool.tile([128, 128], f32, name="ones_f")
            nc.gpsimd.memset(ones_f[:], 1.0)
            ones_b = cpool.tile([128, 128], bf16, name="ones_b")
            nc.vector.tensor_copy(ones_b[:], ones_f[:])
            onecol_f = cpool.tile([128, 1], f32, name="onecol_f")
            nc.gpsimd.memset(onecol_f[:], 1.0)

            bqr_s = cpool.tile([128, FT], f32, name="bqr_s")
            nc.sync.dma_start(bqr_s[:], bqr_d[:])
            bkr_s = cpool.tile([128, FT], f32, name="bkr_s")
            nc.sync.dma_start(bkr_s[:], bkr_d[:])
            bv_r = cpool.tile([1, HD], bf16, name="bv_r")
            nc.sync.dma_start(bv_r[:], bv_d[:])

            # persistent SBUF arrays
            K = big.tile([128, FT, S], bf16, name="Kfm")
            Vs = big.tile([128, RT, 8, 65], bf16, name="Vs")
            X = big.tile([128, FT, S], bf16, name="Xfm")
            wk_s = big.tile([128, KC, HD], bf16, name="wk_s")
            wv_s = big.tile([128, KC, HD], bf16, name="wv_s")
            wq_s = big.tile([128, KC, HD], bf16, name="wq_s")
            wo_s = big.tile([128, FT, D], bf16, name="wo_s")
            xk_sb = big.tile([128, KC, S], bf16, name="xk_sb")
            xv_sb = big.tile([128, KC, S], bf16, name="xv_sb")

            # ones column of the augmented V
            nc.vector.tensor_copy(
                Vs[:, :, :, 64:65],
                onecol_f[:, 0:1].to_broadcast((128, RT, 8, 1)),
            )

            # weight/x DMAs. sync queue: K-projection critical path.
            wk_src = wk_d[:].rearrange("(k p) n -> p k n", p=128)
            for kc in range(KC):
                nc.sync.dma_start(wk_s[:, kc, :], wk_src[:, kc, :])
            xk_src = xk_d[:].rearrange("(k p) s -> p k s", p=128)
            xv_src = xv_d[:].rearrange("(k p) s -> p k s", p=128)
            for kc in range(KC):
                eng = nc.sync if kc % 2 == 0 else nc.gpsimd
                eng.dma_start(xk_sb[:, kc, :], xk_src[:, kc, :])
            # gpsimd queue: wq early (Qt(0,0) needed at attention start)
            wq_src = wq_d[:].rearrange("(k p) n -> p k n", p=128)
            for kc in range(KC):
                nc.gpsimd.dma_start(wq_s[:, kc, :], wq_src[:, kc, :])
            wv_src = wv_d[:].rearrange("(k p) n -> p k n", p=128)
            for kc in range(KC):
                nc.gpsimd.dma_start(wv_s[:, kc, :], wv_src[:, kc, :])
            for kc in range(KC):
                eng = nc.sync if kc % 2 == 0 else nc.gpsimd
                eng.dma_start(xv_sb[:, kc, :], xv_src[:, kc, :])
            wo_src = wo_d[:].rearrange("(f p) n -> p f n", p=128)
            for fc in range(FT):
                nc.gpsimd.dma_start(wo_s[:, fc, :], wo_src[:, fc, :])

            with (
                tc.tile_pool(name="xqp", bufs=2) as xqp,
                tc.tile_pool(name="qtp", bufs=3) as qtp,
                tc.tile_pool(name="ptp", bufs=6) as ptp,
                tc.tile_pool(name="rivp", bufs=4) as rivp,
                tc.tile_pool(name="bcp", bufs=2) as bcp,
                tc.tile_pool(name="ostage", bufs=4) as ostage,
                tc.tile_pool(name="rsd", bufs=2, space="DRAM") as rsd,
                tc.tile_pool(name="psc", bufs=2, space="PSUM") as psc,
                tc.tile_pool(name="px", bufs=2, space="PSUM") as px,
                tc.tile_pool(name="pqo", bufs=2, space="PSUM") as pqo,
            ):
                xq_src = xq_d[:].rearrange("(k p) s -> p k s", p=128)

                def stage_xq(t):
                    xt = xqp.tile([128, KC, 512], bf16, tag="xq", name="xqt")
                    tsl = slice(t * 512, (t + 1) * 512)
                    for kc in range(KC):
                        nc.sync.dma_start(xt[:, kc, :], xq_src[:, kc, tsl])
                    return xt

                xq_tiles = {0: stage_xq(0)}

                def emit_kproj(ft, h):
                    hsl = slice(h * 512, (h + 1) * 512)
                    ps = pqo.tile([128, 512], f32, tag="qo", name="pk")
                    for kc in range(KC):
                        nc.tensor.matmul(
                            ps[:],
                            wk_s[:, kc, ft * 128 : (ft + 1) * 128],
                            xk_sb[:, kc, hsl],
                            start=(kc == 0),
                            stop=(kc == KC - 1),
                        )
                    nc.vector.tensor_scalar_add(
                        K[:, ft, hsl], ps[:], bkr_s[:, ft : ft + 1]
                    )

                def emit_qt(t, j):
                    qp = pqo.tile([128, 512], f32, tag="qo", name="qp")
                    xt = xq_tiles[t]
                    for kc in range(KC):
                        nc.tensor.matmul(
                            qp[:],
                            wq_s[:, kc, j * 128 : (j + 1) * 128],
                            xt[:, kc, :],
                            start=(kc == 0),
                            stop=(kc == KC - 1),
                        )
                    qt = qtp.tile([128, 512], bf16, tag="qt", name="qt")
                    nc.vector.tensor_scalar_add(
                        qt[:], qp[:], bqr_s[:, j : j + 1]
                    )
                    return qt

                def emit_vproj(rt):
                    ps = pqo.tile([128, 512], f32, tag="qo", name="vp")
                    for kc in range(KC):
                        nc.tensor.matmul(
                            ps[:],
                            xv_sb[:, kc, rt * 128 : (rt + 1) * 128],
                            wv_s[:, kc, :],
                            start=(kc == 0),
                            stop=False,
                        )
                    nc.tensor.matmul(
                        ps[:],
                        ones_b[0:1, :],
                        bv_r[0:1, :],
                        start=False,
                        stop=True,
                    )
                    nc.vector.tensor_copy(
                        Vs[:, rt, :, 0:64],
                        ps[:].rearrange("p (h e) -> p h e", h=8),
                    )

                def emit_outproj_piece(t, r2, n):
                    rt = t * 4 + r2
                    rsl = slice(rt * 128, (rt + 1) * 128)
                    ps = pqo.tile([128, 512], f32, tag="qo", name="po")
                    for fc in range(FT):
                        nc.tensor.matmul(
                            ps[:],
                            X[:, fc, rsl],
                            wo_s[:, fc, n * 512 : (n + 1) * 512],
                            start=(fc == 0),
                            stop=(fc == FT - 1),
                        )
                    ot = ostage.tile([128, 512], f32, tag="os", name="os")
                    nc.vector.tensor_copy(ot[:], ps[:])
                    nc.sync.dma_start(
                        o_d[rsl, n * 512 : (n + 1) * 512], ot[:]
                    )

                # phase 1: full K projection, V 0-2, Qt(0,0)
                for ft in range(FT):
                    for h in range(4):
                        emit_kproj(ft, h)
                qts = {(0, 0): emit_qt(0, 0)}
                for rt in range(3):
                    emit_vproj(rt)

                for t in range(ST):
                    tsl = slice(t * 512, (t + 1) * 512)
                    for j in range(FT):
                        qt = qts.pop((t, j))
                        xpA = px.tile([65, 512], f32, tag="px", name="xpA")
                        xpB = px.tile([65, 512], f32, tag="px", name="xpB")

                        # filler inventory for this (t, j) window
                        fillers = []
                        if t > 0 and j == 0:
                            for r2 in range(4):
                                for n in range(2):
                                    fillers.append(("op", (t - 1, r2, n)))
                        if j < FT - 1:
                            fillers.append(("qt", (t, j + 1)))
                        else:
                            if t < ST - 1:
                                fillers.append(("qt", (t + 1, 0)))
                        if j == 2 and t < ST - 1:
                            fillers.append(("xq", t + 1))

                        def pop_filler():
                            if not fillers:
                                return
                            kind, arg = fillers.pop(0)
                            if kind == "qt":
                                qts[arg] = emit_qt(*arg)
                            elif kind == "xq":
                                xq_tiles[arg] = stage_xq(arg)
                            elif kind == "op":
                                emit_outproj_piece(*arg)

                        if t == 0 and j == 0:
                            fillers.append(("qt", (0, 1)))

                        for k in range(RT):
                            ksl = slice(k * 128, (k + 1) * 128)
                            sAB = psc.tile([128, 2, 512], f32, tag="sc",
                                           name="sAB")
                            nc.tensor.matmul(
                                sAB[:, 0, :], K[0:64, j, ksl],
                                qt[0:64, :],
                                start=True, stop=True,
                                tile_position=(0, 0),
                            )
                            nc.tensor.matmul(
                                sAB[:, 1, :], K[64:128, j, ksl],
                                qt[64:128, :],
                                start=True, stop=True,
                                tile_position=(64, 0),
                            )
                            pAB = ptp.tile([128, 2, 512], bf16, tag="pt",
                                           name="pAB")
                            nc.scalar.activation(
                                pAB[:], sAB[:], EXP, scale=0.125)
                            if DEBUG_DUMPS and t == 0 and j == 0 and k == 0:
                                nc.gpsimd.dma_start(dbg_p[:], pAB[:])

                            # V chunks just-in-time, 3 ahead of the AV
                            if t == 0 and j == 0 and k + 3 < RT:
                                emit_vproj(k + 3)

                            # AV with ones-augmented V: X and row sums
                            st = (k == 0)
                            sp = (k == RT - 1)
                            nc.tensor.matmul(
                                xpA[:], Vs[:, k, 2 * j, :],
                                pAB[:, 0, :], start=st, stop=sp,
                            )
                            nc.tensor.matmul(
                                xpB[:], Vs[:, k, 2 * j + 1, :],
                                pAB[:, 1, :], start=st, stop=sp,
                            )

                            pop_filler()

                        while fillers:
                            pop_filler()

                        # normalize: drain row sums to SBUF, reciprocal,
                        # DMA round-trip broadcast, deferred multiply
                        rsj = rivp.tile([33, 512], f32, tag="rs", name="rsj")
                        nc.vector.tensor_copy(rsj[0:1, :], xpA[64:65, :])
                        nc.vector.tensor_copy(rsj[32:33, :], xpB[64:65, :])
                        riv = rivp.tile([33, 512], f32, tag="ri", name="riv")
                        nc.vector.reciprocal_approx_fast(riv[:], rsj[:])
                        if DEBUG_DUMPS and t == 0 and j == 0:
                            nc.gpsimd.dma_start(dbg_rs[0:1, :], rsj[0:1, :])
                            nc.gpsimd.dma_start(dbg_rs[1:2, :], rsj[32:33, :])
                            nc.gpsimd.dma_start(dbg_ri[0:1, :], riv[0:1, :])
                            nc.gpsimd.dma_start(dbg_ri[1:2, :], riv[32:33, :])
                            nc.gpsimd.dma_start(dbg_qt[:], qt[:])
                        nc.vector.tensor_copy(
                            X[0:64, j, tsl], xpA[0:64, :])
                        nc.vector.tensor_copy(
                            X[64:128, j, tsl], xpB[0:64, :])
                        rd = rsd.tile([2, 512], f32, tag="rd", name="rd")
                        nc.sync.dma_start(rd[0:1, :], riv[0:1, :])
                        nc.sync.dma_start(rd[1:2, :], riv[32:33, :])
                        bcs = bcp.tile([128, 512], f32, tag="bcs",
                                       name="bcs")
                        nc.sync.dma_start(
                            bcs[0:64, :],
                            rd[0:1, :].to_broadcast((64, 512)))
                        nc.sync.dma_start(
                            bcs[64:128, :],
                            rd[1:2, :].to_broadcast((64, 512)))
                        nc.vector.tensor_mul(
                            X[:, j, tsl], X[:, j, tsl], bcs[:])

                # final out projection for t=3
                for r2 in range(4):
                    for n in range(2):
                        emit_outproj_piece(ST - 1, r2, n)

                if DEBUG_DUMPS:
                    nc.sync.dma_start(
                        dbg_xk[:], xk_sb[:].rearrange("p a b -> p (a b)"))
                    nc.sync.dma_start(
                        dbg_wk[:], wk_s[:].rearrange("p a b -> p (a b)"))
                    nc.sync.dma_start(
                        dbg_k[:], K[:].rearrange("p a b -> p (a b)"))
                    nc.sync.dma_start(
                        dbg_v[:], Vs[:].rearrange("p a b c -> p (a b c)"))
                    nc.sync.dma_start(
                        dbg_x[:], X[:].rearrange("p a b -> p (a b)"))

    nc.compile()
    return nc


def kernel(**inputs):
    global _CACHED_NC, _LAST_IN_MAPS
    if _CACHED_NC is None:
        _CACHED_NC = build_nc()
    nc = _CACHED_NC

    bf = ml_dtypes.bfloat16
    query = np.asarray(inputs["query"], dtype=np.float32)
    key = np.asarray(inputs["key"], dtype=np.float32)
    value = np.asarray(inputs["value"], dtype=np.float32)
    fc_w = np.asarray(inputs["fc_w"], dtype=np.float32)
    Wq = np.asarray(inputs["Wq"], dtype=np.float32)
    Wk = np.asarray(inputs["Wk"], dtype=np.float32)
    Wv = np.asarray(inputs["Wv"], dtype=np.float32)
    Wo = np.asarray(inputs["Wo"], dtype=np.float32)
    bq = np.asarray(inputs["bq"], dtype=np.float32)
    bk = np.asarray(inputs["bk"], dtype=np.float32)
    bv = np.asarray(inputs["bv"], dtype=np.float32)
    bo = np.asarray(inputs["bo"], dtype=np.float32)

    wq_eff = (fc_w * Wq).astype(bf)
    wk_b = Wk.astype(bf)
    wv_b = Wv.astype(bf)
    wo_b = Wo.astype(bf)
    xs = {}
    for b in range(B):
        xs[b] = (
            np.ascontiguousarray(query[b].T.astype(bf)),
            np.ascontiguousarray(key[b].T.astype(bf)),
            np.ascontiguousarray(value[b].T.astype(bf)),
        )

    in_maps = []
    for c in range(8):
        b, hh = c // 2, c % 2
        hs = slice(hh * HD, (hh + 1) * HD)
        xq, xk, xv = xs[b]
        in_maps.append({
            "xq": xq,
            "xk": xk,
            "xv": xv,
            "wq": np.ascontiguousarray(wq_eff[:, hs]),
            "wk": np.ascontiguousarray(wk_b[:, hs]),
            "wv": np.ascontiguousarray(wv_b[:, hs]),
            "wo": np.ascontiguousarray(wo_b[hs, :]),
            "bqr": np.ascontiguousarray(bq[hs].reshape(FT, 128).T),
            "bkr": np.ascontiguousarray(bk[hs].reshape(FT, 128).T),
            "bv": bv[None, hs].astype(bf),
        })

    _LAST_IN_MAPS = in_maps
    res = run_bass_kernel_spmd(nc, in_maps, core_ids=list(range(8)))

    out = np.empty((B, S, D), dtype=np.float32)
    for b in range(B):
        out[b] = res.results[2 * b]["o"] + res.results[2 * b + 1]["o"] + bo
    return out
